# revision 1
# baseline (speedup 1.0000x reference)
"""Trainium2 Bass kernel for nn_MixedGatedMLP (4-bit quantized gated MLP + LoRA).

Strategy: tensor-parallel over d_ff across 8 NeuronCores (F padded 11008->11264,
1408 rows/core). Each core dequantizes its 4-bit weight shards on-device
(nibble extract on DVE, u8->bf16 convert on ACT, 16-entry codebook lookup as
is_equal/mult masked accumulation, blockwise absmax scale), runs the three
matmuls in bf16 on TensorE with LoRA folded in as extra contraction rows,
applies silu-gating, and the partial down-proj outputs are exchanged with an
AllToAll + local adds (reduce-scatter equivalent at copy rate). Core i returns
final tokens [512*i, 512*(i+1)); the host concatenates.
"""

import sys

for _p in ("/opt/trn_rl_repo", "/root/.axon_site/_ro/trn_rl_repo"):
    if _p not in sys.path:
        sys.path.append(_p)

from contextlib import ExitStack

import numpy as np
import ml_dtypes

import concourse.bass as bass
import concourse.mybir as mybir
import concourse.tile as tile
from concourse import bacc
from concourse.bass_utils import run_bass_kernel_spmd

BF16 = ml_dtypes.bfloat16
NCORES = 8
ALU = mybir.AluOpType
AFT = mybir.ActivationFunctionType


class Cfg:
    def __init__(self, D=4096, T=4096, F=11008, R=16, block=64, ncores=8):
        self.D = D              # d_model
        self.T = T              # tokens
        self.F = F              # true d_ff
        self.R = R              # lora rank
        self.block = block      # absmax block size
        self.ncores = ncores
        # padded d_ff: per-core shard must be a multiple of 2*block so the
        # down-proj byte/absmax slicing stays aligned
        unit = 2 * block * ncores
        self.FP = ((F + unit - 1) // unit) * unit
        self.FS = self.FP // ncores          # per-core f rows
        self.TS = T // ncores                # per-core output tokens
        self.DP = D // 256                   # 256-d chunks (128 byte-pairs)
        self.NT = T // 512                   # 512-token tiles
        # phase-1 f slices (512 wide)
        self.f_slices = []
        f0 = 0
        while f0 < self.FS:
            w = min(512, self.FS - f0)
            self.f_slices.append((f0, w))
            f0 += w
        # phase-2 dd quarters
        self.DDQ = max(512, D // 4)
        self.n_q = D // self.DDQ
        self.NTG = T // 128                  # 128-token groups
        self.NFG = self.FS // 128            # 128-f groups
        # down byte-pair chunks (pairs of f): [(j0, j1), ...] <=128 each
        self.j_chunks = []
        j0 = 0
        npairs = self.FS // 2
        while j0 < npairs:
            j1 = min(j0 + 128, npairs)
            self.j_chunks.append((j0, j1))
            j0 = j1
        # which merge-adds go to gpsimd (k indices in 2..15)
        self.gp_adds = set()
        # HW has a fused Silu activation; CoreSim only implements Sigmoid
        self.use_silu = True


def _dperm(D):
    """Row order of xT: per 256-d chunk, evens then odds."""
    idx = []
    for c in range(D // 256):
        base = 256 * c
        idx.extend(range(base, base + 256, 2))
        idx.extend(range(base + 1, base + 256, 2))
    return np.array(idx)


def _fperm_local(cfg):
    """Within-shard f order: per down j-chunk, even f (2j) then odd f (2j+1)."""
    idx = []
    for (j0, j1) in cfg.j_chunks:
        idx.extend(2 * j for j in range(j0, j1))
        idx.extend(2 * j + 1 for j in range(j0, j1))
    return np.array(idx)


def build_graph(cfg: Cfg):
    nc = bacc.Bacc(None, num_devices=cfg.ncores)
    dt = mybir.dt
    D, T, FS, R = cfg.D, cfg.T, cfg.FS, cfg.R

    # ---- external inputs (per-core) ----
    xT = nc.dram_tensor("xT", [D, T], dt.bfloat16, kind="ExternalInput")
    g_bytes = nc.dram_tensor("g_bytes", [D // 2, FS], dt.uint8, kind="ExternalInput")
    u_bytes = nc.dram_tensor("u_bytes", [D // 2, FS], dt.uint8, kind="ExternalInput")
    d_bytes = nc.dram_tensor("d_bytes", [FS // 2, D], dt.uint8, kind="ExternalInput")
    g_am = nc.dram_tensor("g_am", [D // 2, FS], dt.bfloat16, kind="ExternalInput")
    u_am = nc.dram_tensor("u_am", [D // 2, FS], dt.bfloat16, kind="ExternalInput")
    d_am = nc.dram_tensor("d_am", [FS // 2, D], dt.bfloat16, kind="ExternalInput")
    code_rep = nc.dram_tensor("code_rep", [128, 16], dt.float32, kind="ExternalInput")
    a_gu = nc.dram_tensor("a_gu", [D, 2 * R], dt.bfloat16, kind="ExternalInput")
    b_g = nc.dram_tensor("b_g", [R, FS], dt.bfloat16, kind="ExternalInput")
    b_u = nc.dram_tensor("b_u", [R, FS], dt.bfloat16, kind="ExternalInput")
    a_d = nc.dram_tensor("a_d", [FS, R], dt.bfloat16, kind="ExternalInput")
    b_d = nc.dram_tensor("b_d", [R, D], dt.bfloat16, kind="ExternalInput")

    y_out = nc.dram_tensor("y_out", [cfg.TS, D], dt.float32, kind="ExternalOutput")

    # ---- internal DRAM ----
    x3_dram = nc.dram_tensor("x3_dram", [FS, T], dt.bfloat16, kind="Internal")
    xag_dram = nc.dram_tensor("xag_dram", [R, T], dt.bfloat16, kind="Internal")
    xau_dram = nc.dram_tensor("xau_dram", [R, T], dt.bfloat16, kind="Internal")
    x3a_dram = nc.dram_tensor("x3a_dram", [R, T], dt.bfloat16, kind="Internal")
    a2a_in = [
        nc.dram_tensor(f"a2a_in{i}", [T, cfg.DDQ], dt.bfloat16, kind="Internal")
        for i in range(2)
    ]
    a2a_out = [
        nc.dram_tensor(f"a2a_out{i}", [T, cfg.DDQ], dt.bfloat16, kind="Internal")
        for i in range(2)
    ]

    rg = [list(range(cfg.ncores))]

    with tile.TileContext(nc) as tc, ExitStack() as ctx:
        const_pool = ctx.enter_context(tc.tile_pool(name="const", bufs=1))
        code_sb = const_pool.tile([128, 16], dt.float32)
        nc.sync.dma_start(code_sb[:], code_rep[:])
        agu_sb = const_pool.tile([128, D // 128, 2 * R], dt.bfloat16)
        nc.sync.dma_start(
            agu_sb[:], a_gu.rearrange("(c p) r -> p c r", p=128)
        )
        bg_sb = const_pool.tile([R, FS], dt.bfloat16)
        nc.sync.dma_start(bg_sb[:], b_g[:])
        bu_sb = const_pool.tile([R, FS], dt.bfloat16)
        nc.sync.dma_start(bu_sb[:], b_u[:])
        ad_sb = const_pool.tile([128, FS // 128, R], dt.bfloat16)
        nc.sync.dma_start(ad_sb[:], a_d.rearrange("(c p) r -> p c r", p=128))
        dq_pool = ctx.enter_context(tc.tile_pool(name="dq", bufs=3))
        wd_pool = ctx.enter_context(tc.tile_pool(name="wd", bufs=cfg.NFG + 2))

        def dequant_chunk(bytes_dram, am_dram, p0, pc, fsel0, fw, outs, pool,
                          pbase=0):
            """Dequant byte rows [p0:p0+pc] x cols [fsel0:fsel0+fw] of a packed
            tensor into outs = (W_hi, W_lo) bf16 AP tiles of shape [pc, fw],
            processing on SBUF partitions [pbase, pbase+pc). outs entries may
            be None to skip that nibble."""
            psl = slice(pbase, pbase + pc)
            B = pool.tile([128, fw], dt.uint8, tag="bq")
            S = pool.tile([128, fw], dt.bfloat16, tag="sq")
            nc.sync.dma_start(B[psl, :], bytes_dram[p0:p0 + pc, fsel0:fsel0 + fw])
            nc.sync.dma_start(S[psl, :], am_dram[p0:p0 + pc, fsel0:fsel0 + fw])
            plan = []
            if outs[0] is not None:
                plan.append((4, ALU.logical_shift_right, outs[0]))
            if outs[1] is not None:
                plan.append((15, ALU.bitwise_and, outs[1]))
            for sc, op, W in plan:
                U = pool.tile([128, fw], dt.uint8, tag="u")
                nc.vector.tensor_scalar(U[psl, :], B[psl, :], sc, None, op)
                X = pool.tile([128, fw], dt.bfloat16, tag="x")
                nc.scalar.copy(X[psl, :], U[psl, :])
                acc0 = pool.tile([128, fw], dt.bfloat16, tag="a0")
                acc1 = pool.tile([128, fw], dt.bfloat16, tag="a1")
                tk = pool.tile([128, fw], dt.bfloat16, tag="tk")
                nc.vector.tensor_scalar(acc0[psl, :], X[psl, :], 0.0,
                                        code_sb[psl, 0:1], ALU.is_equal, ALU.mult)
                nc.vector.tensor_scalar(acc1[psl, :], X[psl, :], 1.0,
                                        code_sb[psl, 1:2], ALU.is_equal, ALU.mult)
                for k in range(2, 16):
                    acc = acc0 if (k % 2 == 0) else acc1
                    nc.vector.tensor_scalar(tk[psl, :], X[psl, :], float(k),
                                            code_sb[psl, k:k + 1],
                                            ALU.is_equal, ALU.mult)
                    eng = nc.gpsimd if k in cfg.gp_adds else nc.vector
                    eng.tensor_tensor(acc[psl, :], acc[psl, :], tk[psl, :], ALU.add)
                nc.vector.tensor_tensor(acc0[psl, :], acc0[psl, :], acc1[psl, :],
                                        ALU.add)
                nc.vector.tensor_tensor(W, acc0[psl, :], S[psl, :], ALU.mult)

        # =============== phase 1: gate/up matmuls -> x3 ===============
        with (
            tc.tile_pool(name="w", bufs=cfg.DP + 1) as w_pool,
            tc.tile_pool(name="xt", bufs=2 * cfg.DP + 2) as xt_pool,
            tc.tile_pool(name="p1", bufs=3) as p1_pool,
            tc.tile_pool(name="ps1", bufs=2, space="PSUM") as psum1,
            tc.tile_pool(name="psa", bufs=1, space="PSUM") as psuma,
        ):
            for (f0, fw) in cfg.f_slices:
                wg = {}
                wu = {}
                for c in range(cfg.DP):
                    for name, bsrc, asrc, wdict in (
                        ("g", g_bytes, g_am, wg), ("u", u_bytes, u_am, wu),
                    ):
                        th = w_pool.tile([128, fw], dt.bfloat16, tag=f"w{name}h")
                        tl = w_pool.tile([128, fw], dt.bfloat16, tag=f"w{name}l")
                        dequant_chunk(bsrc, asrc, 128 * c, 128, f0, fw,
                                      (th[:, :], tl[:, :]), dq_pool)
                        wdict[2 * c] = th
                        wdict[2 * c + 1] = tl
                for t in range(cfg.NT):
                    tt = slice(512 * t, 512 * (t + 1))
                    xts = []
                    for ci in range(2 * cfg.DP):
                        xt_t = xt_pool.tile([128, 512], dt.bfloat16, tag="xt")
                        nc.sync.dma_start(
                            xt_t[:], xT[128 * ci:128 * (ci + 1), tt]
                        )
                        xts.append(xt_t)
                    if f0 == 0:
                        # x@Ag, x@Au for this token tile
                        for ri, dst in ((0, xag_dram), (1, xau_dram)):
                            pa = psuma.tile([R, 512], dt.float32, tag="pa")
                            for ci in range(2 * cfg.DP):
                                nc.tensor.matmul(
                                    pa[:], agu_sb[:, ci, R * ri:R * (ri + 1)],
                                    xts[ci][:],
                                    start=(ci == 0), stop=(ci == 2 * cfg.DP - 1))
                            st = p1_pool.tile([R, 512], dt.bfloat16, tag="st")
                            nc.scalar.copy(st[:], pa[:])
                            nc.sync.dma_start(dst[:, tt], st[:])
                    xag_t = p1_pool.tile([R, 512], dt.bfloat16, tag="xag_t")
                    nc.sync.dma_start(xag_t[:], xag_dram[:, tt])
                    xau_t = p1_pool.tile([R, 512], dt.bfloat16, tag="xau_t")
                    nc.sync.dma_start(xau_t[:], xau_dram[:, tt])
                    for g in range(fw // 128):
                        fg = slice(128 * g, 128 * (g + 1))
                        fga = slice(f0 + 128 * g, f0 + 128 * (g + 1))
                        pg = psum1.tile([128, 512], dt.float32, tag="pg")
                        pu = psum1.tile([128, 512], dt.float32, tag="pu")
                        for ci in range(2 * cfg.DP):
                            nc.tensor.matmul(pg[:], wg[ci][:, fg], xts[ci][:],
                                             start=(ci == 0), stop=False)
                        nc.tensor.matmul(pg[:], bg_sb[:, fga], xag_t[:],
                                         start=False, stop=True)
                        for ci in range(2 * cfg.DP):
                            nc.tensor.matmul(pu[:], wu[ci][:, fg], xts[ci][:],
                                             start=(ci == 0), stop=False)
                        nc.tensor.matmul(pu[:], bu_sb[:, fga], xau_t[:],
                                         start=False, stop=True)
                        sg = p1_pool.tile([128, 512], dt.bfloat16, tag="sg")
                        if cfg.use_silu:
                            nc.scalar.activation(sg[:], pg[:], AFT.Silu)
                        else:
                            nc.scalar.activation(sg[:], pg[:], AFT.Sigmoid)
                            nc.vector.tensor_tensor(sg[:], sg[:], pg[:], ALU.mult)
                        x3t = p1_pool.tile([128, 512], dt.bfloat16, tag="x3t")
                        nc.vector.tensor_tensor(x3t[:], sg[:], pu[:], ALU.mult)
                        nc.sync.dma_start(x3_dram[fga, tt], x3t[:])

        # =============== phase 2: down matmul + A2A reduce ===============
        n_dd = cfg.DDQ // 512
        with (
            tc.tile_pool(name="p2", bufs=4) as p2_pool,
            tc.tile_pool(name="lor", bufs=3) as lor_pool,
            tc.tile_pool(name="red", bufs=cfg.ncores + 2) as red_pool,
            tc.tile_pool(name="yfp", bufs=2) as yf_pool,
            tc.tile_pool(name="ps2", bufs=2, space="PSUM") as psum2,
            tc.tile_pool(name="psb", bufs=1, space="PSUM") as psumb,
        ):
            # x3 @ Ad -> x3a_dram
            for t in range(cfg.NT):
                tt = slice(512 * t, 512 * (t + 1))
                pa = psumb.tile([R, 512], dt.float32, tag="pa2")
                for g in range(cfg.NFG):
                    x3l = p2_pool.tile([128, 512], dt.bfloat16, tag="x3a_in")
                    nc.sync.dma_start(x3l[:], x3_dram[128 * g:128 * (g + 1), tt])
                    nc.tensor.matmul(pa[:], ad_sb[:, g, :], x3l[:],
                                     start=(g == 0), stop=(g == cfg.NFG - 1))
                st2 = lor_pool.tile([R, 512], dt.bfloat16, tag="st2")
                nc.scalar.copy(st2[:], pa[:])
                nc.sync.dma_start(x3a_dram[:, tt], st2[:])

            for q in range(cfg.n_q):
                dd0 = cfg.DDQ * q
                dds = slice(dd0, dd0 + cfg.DDQ)
                wd = {}
                for ic, (j0, j1) in enumerate(cfg.j_chunks):
                    pc = j1 - j0
                    wh = wd_pool.tile([128, cfg.DDQ], dt.bfloat16, tag="wd",
                                      name=f"wdh{ic}")
                    if pc == 128:
                        wl = wd_pool.tile([128, cfg.DDQ], dt.bfloat16, tag="wd",
                                          name=f"wdl{ic}")
                        dequant_chunk(d_bytes, d_am, j0, pc, dd0, cfg.DDQ,
                                      (wh[:, :], wl[:, :]), dq_pool)
                        wd[2 * ic] = wh
                        wd[2 * ic + 1] = wl
                    else:
                        # ragged tail: H rows at partitions [0,pc),
                        # L rows at [pc,2pc) of the same f-group
                        dequant_chunk(d_bytes, d_am, j0, pc, dd0, cfg.DDQ,
                                      (wh[0:pc, :], None), dq_pool, pbase=0)
                        dequant_chunk(d_bytes, d_am, j0, pc, dd0, cfg.DDQ,
                                      (None, wh[pc:2 * pc, :]), dq_pool, pbase=pc)
                        wd[2 * ic] = wh
                bdt = lor_pool.tile([R, cfg.DDQ], dt.bfloat16, tag="bdt")
                nc.sync.dma_start(bdt[:], b_d[:, dds])
                for tg in range(cfg.NTG):
                    tsl = slice(128 * tg, 128 * (tg + 1))
                    x3at = lor_pool.tile([R, 128], dt.bfloat16, tag="x3at")
                    nc.sync.dma_start(x3at[:], x3a_dram[:, tsl])
                    pds = [psum2.tile([128, 512], dt.float32, tag=f"pd{dj}",
                                      name=f"pd{dj}")
                           for dj in range(n_dd)]
                    for g in range(cfg.NFG):
                        x3l = p2_pool.tile([128, 128], dt.bfloat16, tag="x3l")
                        nc.sync.dma_start(x3l[:],
                                          x3_dram[128 * g:128 * (g + 1), tsl])
                        for dj in range(n_dd):
                            nc.tensor.matmul(pds[dj][:], x3l[:],
                                             wd[g][:, 512 * dj:512 * (dj + 1)],
                                             start=(g == 0), stop=False)
                    for dj in range(n_dd):
                        nc.tensor.matmul(
                            pds[dj][:], x3at[:],
                            bdt[:, 512 * dj:512 * (dj + 1)],
                            start=False, stop=True)
                    for dj in range(n_dd):
                        yb = p2_pool.tile([128, 512], dt.bfloat16, tag="yb")
                        nc.scalar.copy(yb[:], pds[dj][:])
                        nc.sync.dma_start(
                            a2a_in[q % 2][tsl, 512 * dj:512 * (dj + 1)], yb[:])
                nc.gpsimd.collective_compute(
                    "AllToAll", ALU.bypass, replica_groups=rg,
                    ins=[a2a_in[q % 2][:, :].opt()],
                    outs=[a2a_out[q % 2][:, :].opt()],
                )
                # local reduce of the 8 received partials for my TS tokens
                tst = min(128, cfg.TS)
                for ts in range(cfg.TS // tst):
                    for dj in range(n_dd):
                        dsl = slice(512 * dj, 512 * (dj + 1))
                        parts = []
                        for j in range(cfg.ncores):
                            pt = red_pool.tile([128, 512], dt.bfloat16, tag="rp")
                            r0 = cfg.TS * j + tst * ts
                            nc.sync.dma_start(pt[:tst, :],
                                              a2a_out[q % 2][r0:r0 + tst, dsl])
                            parts.append(pt)
                        for lvl in (4, 2):
                            for j in range(lvl):
                                nc.vector.tensor_tensor(parts[j][:tst, :],
                                                        parts[j][:tst, :],
                                                        parts[j + lvl][:tst, :],
                                                        ALU.add)
                        yf = yf_pool.tile([128, 512], dt.float32, tag="yf")
                        nc.vector.tensor_tensor(yf[:tst, :], parts[0][:tst, :],
                                                parts[1][:tst, :], ALU.add)
                        nc.sync.dma_start(
                            y_out[tst * ts:tst * (ts + 1),
                                  dd0 + 512 * dj:dd0 + 512 * (dj + 1)],
                            yf[:tst, :]
                        )

    nc.compile()
    return nc


# ----------------- host side -----------------

_CACHE = {}


def _get_graph(cfg: Cfg):
    key = (cfg.D, cfg.T, cfg.F, cfg.ncores)
    if key not in _CACHE:
        _CACHE[key] = build_graph(cfg)
    return _CACHE[key]


def _prep_inputs(cfg: Cfg, inputs):
    """Shard + lay out the full inputs for each core. Marshalling only."""
    D, T, F, FP, FS, R = cfg.D, cfg.T, cfg.F, cfg.FP, cfg.FS, cfg.R
    blk = cfg.block
    dperm = _dperm(D)
    fperm = _fperm_local(cfg)

    x = inputs["x"]
    xT = np.ascontiguousarray(x.T[dperm]).astype(BF16)

    def pack_rows(packed, absmax):
        """gate/up style: packed [F*D/2] -> per-core (bytes [D/2, FS], am plane)."""
        b = (packed.astype(np.int64) & 0xFF).astype(np.uint8).reshape(F, D // 2)
        b = np.concatenate([b, np.zeros((FP - F, D // 2), np.uint8)], 0)
        am = absmax.reshape(F, D // blk).astype(np.float32)
        am = np.concatenate([am, np.zeros((FP - F, D // blk), np.float32)], 0)
        outs = []
        for i in range(cfg.ncores):
            bs = b[FS * i:FS * (i + 1)][fperm]           # [FS, D/2]
            ams = am[FS * i:FS * (i + 1)][fperm]         # [FS, D/blk]
            bT = np.ascontiguousarray(bs.T)              # [D/2, FS]
            # S plane: S[i_pair, f] = am[f, (2*i_pair)//blk]
            amT = np.repeat(ams.T.astype(BF16), blk // 2, axis=0)  # [D/2, FS]
            outs.append((bT, np.ascontiguousarray(amT)))
        return outs

    def pack_down(packed, absmax):
        """down: packed [D*F/2] -> per-core (bytes [FS/2, D], am plane [FS/2, D])."""
        b = (packed.astype(np.int64) & 0xFF).astype(np.uint8).reshape(D, F // 2)
        b = np.concatenate([b, np.zeros((D, (FP - F) // 2), np.uint8)], 1)
        am = absmax.reshape(D, F // blk).astype(np.float32)
        am = np.concatenate([am, np.zeros((D, (FP - F) // blk), np.float32)], 1)
        outs = []
        npairs = FS // 2
        nblk = FS // blk
        for i in range(cfg.ncores):
            bs = b[:, npairs * i:npairs * (i + 1)]       # [D, FS/2]
            ams = am[:, nblk * i:nblk * (i + 1)]         # [D, FS/blk]
            bT = np.ascontiguousarray(bs.T)              # [FS/2, D]
            amT = np.repeat(ams.T.astype(BF16), blk // 2, axis=0)  # [FS/2, D]
            outs.append((bT, np.ascontiguousarray(amT)))
        return outs

    g = pack_rows(inputs["w_gate_packed"], inputs["w_gate_absmax"])
    u = pack_rows(inputs["w_up_packed"], inputs["w_up_absmax"])
    d = pack_down(inputs["w_down_packed"], inputs["w_down_absmax"])

    code_rep = np.broadcast_to(
        inputs["code"].astype(BF16).astype(np.float32)[None, :], (128, 16)
    ).copy()
    a_gu = np.concatenate(
        [inputs["w_gate_lora_a"], inputs["w_up_lora_a"]], axis=1
    )[dperm].astype(BF16)

    def pad_cols(m):
        return np.concatenate([m, np.zeros((m.shape[0], FP - F), m.dtype)], 1)

    b_g_full = pad_cols(inputs["w_gate_lora_b"].astype(np.float32))
    b_u_full = pad_cols(inputs["w_up_lora_b"].astype(np.float32))
    a_d_full = np.concatenate(
        [inputs["w_down_lora_a"].astype(np.float32),
         np.zeros((FP - F, R), np.float32)], 0
    )
    b_d = inputs["w_down_lora_b"].astype(BF16)

    in_maps = []
    for i in range(cfg.ncores):
        fsl = slice(FS * i, FS * (i + 1))
        in_maps.append({
            "xT": xT,
            "g_bytes": g[i][0], "g_am": g[i][1],
            "u_bytes": u[i][0], "u_am": u[i][1],
            "d_bytes": d[i][0], "d_am": d[i][1],
            "code_rep": code_rep,
            "a_gu": a_gu,
            "b_g": np.ascontiguousarray(b_g_full[:, fsl][:, fperm]).astype(BF16),
            "b_u": np.ascontiguousarray(b_u_full[:, fsl][:, fperm]).astype(BF16),
            "a_d": np.ascontiguousarray(a_d_full[fsl][fperm]).astype(BF16),
            "b_d": b_d,
        })
    return in_maps


def run(cfg: Cfg, inputs, trace=False, **kwargs):
    nc = _get_graph(cfg)
    in_maps = _prep_inputs(cfg, inputs)
    res = run_bass_kernel_spmd(
        nc, in_maps, core_ids=list(range(cfg.ncores)), trace=trace, **kwargs
    )
    y = np.concatenate([res.results[i]["y_out"] for i in range(cfg.ncores)], 0)
    return y, res


def kernel(**inputs) -> np.ndarray:
    cfg = Cfg()
    y, _ = run(cfg, inputs)
    return y.astype(np.float32)



# revision 21
# speedup vs baseline: 1.3042x; 1.3042x over previous
"""Trainium2 Bass kernel for nn_MixedGatedMLP (4-bit quantized gated MLP + LoRA).

Strategy v2: tensor-parallel over d_ff across 8 NeuronCores (F padded
11008->11264, FS=1408 rows/core).  Host ships unpacked nibble planes (u8) and
compact per-64-block absmax rows; the device does the codebook lookup
(16x is_equal/mult tensor_scalar at 4x DVE rate + an add tree partially
offloaded to GPSIMD) and blockwise scaling (scale plane expanded on PE via a
2->128 one-hot matmul, multiplied straight out of PSUM).

Phase 1 streams x in token tiles over ramped f-slices of the gate/up weights;
dequant of slice s+1 overlaps the matmuls of slice s (issue-order zip).
SiLU runs on ACT, the gating multiply on GPSIMD, and x3 goes to DRAM.
The down weights dequant during the last slices' matmuls and round-trip
through DRAM.  Phase 2 recomputes y3 = x3 @ wd + lora in d-halves, with a
token-quartered bf16 ReduceScatter overlapping the next quarter's matmuls.
Core i returns tokens {TQ*q + TQC*i ..} for each quarter q; the host
reassembles.
"""

import sys

for _p in ("/opt/trn_rl_repo", "/root/.axon_site/_ro/trn_rl_repo"):
    if _p not in sys.path:
        sys.path.append(_p)

from contextlib import ExitStack

import numpy as np
import ml_dtypes

import concourse.bass as bass
import concourse.mybir as mybir
import concourse.tile as tile
from concourse import bacc
from concourse.bass_utils import run_bass_kernel_spmd

BF16 = ml_dtypes.bfloat16
ALU = mybir.AluOpType
AFT = mybir.ActivationFunctionType


class Cfg:
    def __init__(self, D=4096, T=4096, F=11008, R=16, block=64, ncores=8,
                 use_silu=True):
        self.D = D
        self.T = T
        self.F = F
        self.R = R
        self.block = block
        self.ncores = ncores
        self.use_silu = use_silu

        unit = 128 * ncores
        self.FP = ((F + unit - 1) // unit) * unit   # padded d_ff
        self.FS = self.FP // ncores                 # per-core f rows
        self.NFG = self.FS // 128                   # 128-row f groups
        self.DCH = D // 128                         # 128-row d chunks
        self.NAB = D // block                       # absmax rows (gate/up)
        self.ABC = 128 // block                     # absmax blocks per chunk

        # token tiling
        self.TTW = min(512, T)                      # phase-1 token tile width
        self.NT = T // self.TTW
        self.NTG = T // 128                         # phase-2 token groups

        # phase-1 f slices in fgroup units: ramped for a small first bubble
        if self.NFG == 11:
            self.slices = [1, 2, 3, 2, 3]
        else:
            self.slices = [1] * self.NFG

        self.gp_terms = 6        # codebook terms handled by gpsimd (g/u)
        self.gp_terms_down = 8   # ... for the down weights (gpsimd idle then)
        self.deq_w = 2048        # target dequant op width (free elems)

        # phase 2
        self.n_dh = 2 if D >= 2048 else 1           # d halves
        self.DH = D // self.n_dh
        self.n_q = 4 if T >= 4096 else 2            # token quarters (RS chunks)
        self.TQ = T // self.n_q
        self.TQC = self.TQ // ncores                # rows per core per quarter
        self.TS = T // ncores

    def slice_groups(self, nfg):
        """Dequant chunk-groups (c0, G) for a slice of nfg fgroups."""
        fw = nfg * 128
        G = max(1, min(self.deq_w // fw, 8, self.DCH))
        while self.DCH % G:
            G -= 1
        return [(c0, G) for c0 in range(0, self.DCH, G)]


def build_graph(cfg: Cfg):
    nc = bacc.Bacc(None, num_devices=cfg.ncores)
    dt = mybir.dt
    D, T, FS, R, NFG = cfg.D, cfg.T, cfg.FS, cfg.R, cfg.NFG
    TTW = cfg.TTW
    rg = [list(range(cfg.ncores))]

    # ---- external inputs (per-core) ----
    xT = nc.dram_tensor("xT", [D, T], dt.bfloat16, kind="ExternalInput")
    g_nib = nc.dram_tensor("g_nib", [D, FS], dt.uint8, kind="ExternalInput")
    u_nib = nc.dram_tensor("u_nib", [D, FS], dt.uint8, kind="ExternalInput")
    d_nib = nc.dram_tensor("d_nib", [FS, D], dt.uint8, kind="ExternalInput")
    g_amc = nc.dram_tensor("g_amc", [cfg.NAB, FS], dt.bfloat16, kind="ExternalInput")
    u_amc = nc.dram_tensor("u_amc", [cfg.NAB, FS], dt.bfloat16, kind="ExternalInput")
    d_amc = nc.dram_tensor("d_amc", [FS // cfg.block, D], dt.bfloat16, kind="ExternalInput")
    code_rep = nc.dram_tensor("code_rep", [128, 16], dt.float32, kind="ExternalInput")
    a_gu = nc.dram_tensor("a_gu", [D, 2 * R], dt.bfloat16, kind="ExternalInput")
    b_g = nc.dram_tensor("b_g", [R, FS], dt.bfloat16, kind="ExternalInput")
    b_u = nc.dram_tensor("b_u", [R, FS], dt.bfloat16, kind="ExternalInput")
    a_d = nc.dram_tensor("a_d", [FS, R], dt.bfloat16, kind="ExternalInput")
    b_d = nc.dram_tensor("b_d", [R, D], dt.bfloat16, kind="ExternalInput")
    eye2 = nc.dram_tensor("eye2", [cfg.ABC, 128], dt.bfloat16, kind="ExternalInput")

    y_out = nc.dram_tensor("y_out", [cfg.TS, D], dt.float32, kind="ExternalOutput")

    # ---- internal DRAM ----
    x3_dram = nc.dram_tensor("x3_dram", [FS, T], dt.bfloat16, kind="Internal")
    wd_dram = nc.dram_tensor("wd_dram", [FS, D], dt.bfloat16, kind="Internal")
    rs_in = [
        nc.dram_tensor(f"rs_in{i}", [cfg.TQ, cfg.DH], dt.bfloat16, kind="Internal")
        for i in range(2)
    ]
    rs_out = [
        nc.dram_tensor(f"rs_out{i}", [cfg.TQC, cfg.DH], dt.bfloat16,
                       kind="Internal")
        for i in range(2)
    ]

    with tile.TileContext(nc) as tc, ExitStack() as ctx:
        # ---------------- constants ----------------
        cpool = ctx.enter_context(tc.tile_pool(name="const", bufs=1))
        code_sb = cpool.tile([128, 16], dt.float32)
        nc.sync.dma_start(code_sb[:], code_rep[:])
        eye2_sb = cpool.tile([cfg.ABC, 128], dt.bfloat16)
        nc.sync.dma_start(eye2_sb[:], eye2[:])
        agu_sb = cpool.tile([128, cfg.DCH, 2 * R], dt.bfloat16)
        nc.sync.dma_start(agu_sb[:], a_gu.rearrange("(c p) r -> p c r", p=128))
        bg_sb = cpool.tile([R, FS], dt.bfloat16)
        nc.sync.dma_start(bg_sb[:], b_g[:])
        bu_sb = cpool.tile([R, FS], dt.bfloat16)
        nc.sync.dma_start(bu_sb[:], b_u[:])
        ad_sb = cpool.tile([128, NFG, R], dt.bfloat16)
        nc.sync.dma_start(ad_sb[:], a_d.rearrange("(c p) r -> p c r", p=128))
        xag_sb = cpool.tile([R, T], dt.bfloat16)
        xau_sb = cpool.tile([R, T], dt.bfloat16)
        x3a_sb = cpool.tile([R, T], dt.bfloat16)

        dqp_cm = tc.tile_pool(name="dq", bufs=1)
        dqp = dqp_cm.__enter__()

        # ------------- phase 1 + dequant -------------
        wtiles = {}          # si -> (wg, wu); even/odd parity tags share bufs
        tasks = []

        def pop_tasks(n):
            for _ in range(n):
                if tasks:
                    tasks.pop(0)()

        with tc.tile_pool(name="w", bufs=1) as wp, \
             tc.tile_pool(name="xt", bufs=1) as xtp, \
             tc.tile_pool(name="p1", bufs=1) as p1p, \
             tc.tile_pool(name="ps1", bufs=1, space="PSUM") as psp:

            def open_wpool(si):
                if si in wtiles:
                    return
                fw = cfg.slices[si] * 128
                par = "eo"[si % 2]
                wg = wp.tile([128, cfg.DCH, fw], dt.bfloat16, tag=f"wg{par}",
                             name=f"wg{si}")
                wu = wp.tile([128, cfg.DCH, fw], dt.bfloat16, tag=f"wu{par}",
                             name=f"wu{si}")
                wtiles[si] = (wg, wu)

            def emit_deq(nib, amc, c0, G, f0, fw, wview, gp, out_dram=None):
                """Dequant chunk rows [128*c0, 128*(c0+G)) x cols [f0, f0+fw)
                into bf16 weights at `wview` ([128, G, fw] AP); optionally DMA
                to out_dram."""
                GW = G * fw
                NB = dqp.tile([128, G, fw], dt.uint8, tag="nb", bufs=2,
                              name="nb")
                nc.sync.dma_start(
                    NB[:], nib[128 * c0:128 * (c0 + G), f0:f0 + fw]
                    .rearrange("(g p) f -> p g f", p=128))
                AM = dqp.tile([cfg.ABC, G, fw], dt.bfloat16, tag="am", bufs=1,
                              name="am")
                nc.sync.dma_start(
                    AM[:], amc[cfg.ABC * c0:cfg.ABC * (c0 + G), f0:f0 + fw]
                    .rearrange("(g b) f -> b g f", b=cfg.ABC))
                X = dqp.tile([128, G, fw], dt.bfloat16, tag="x", bufs=2,
                             name="x")
                nc.scalar.copy(X[:], NB[:])           # ACT: u8 -> bf16
                Xf = X[:].rearrange("p g f -> p (g f)")
                acc0 = dqp.tile([128, GW], dt.bfloat16, tag="a0", name="a0")
                acc1 = dqp.tile([128, GW], dt.bfloat16, tag="a1", name="a1")
                if GW < 1024:
                    gp = 0
                accg = None
                if gp:
                    accg = dqp.tile([128, GW], dt.bfloat16, tag="ag",
                                    name="ag")
                nterm = 0
                for k in range(16):
                    sc2 = code_sb[:, k:k + 1]
                    if k == 0:
                        nc.vector.tensor_scalar(acc0[:], Xf, 0.0, sc2,
                                                ALU.is_equal, ALU.mult)
                    elif k == 1:
                        nc.vector.tensor_scalar(acc1[:], Xf, 1.0, sc2,
                                                ALU.is_equal, ALU.mult)
                    elif gp and k == 2:
                        nc.vector.tensor_scalar(accg[:], Xf, 2.0, sc2,
                                                ALU.is_equal, ALU.mult)
                    elif gp and k < 2 + gp:
                        tg = dqp.tile([128, GW], dt.bfloat16, tag="tg",
                                      bufs=4, name="tg")
                        nc.vector.tensor_scalar(tg[:], Xf, float(k), sc2,
                                                ALU.is_equal, ALU.mult)
                        nc.gpsimd.tensor_tensor(accg[:], accg[:], tg[:],
                                                ALU.add)
                    else:
                        tk = dqp.tile([128, GW], dt.bfloat16, tag="tk",
                                      bufs=1, name="tk")
                        nc.vector.tensor_scalar(tk[:], Xf, float(k), sc2,
                                                ALU.is_equal, ALU.mult)
                        acc = acc0 if (nterm % 2 == 0) else acc1
                        nterm += 1
                        nc.vector.tensor_tensor(acc[:], acc[:], tk[:], ALU.add)
                nc.vector.tensor_tensor(acc0[:], acc0[:], acc1[:], ALU.add)
                if gp:
                    nc.vector.tensor_tensor(acc0[:], acc0[:], accg[:], ALU.add)
                # scale: expand AM rows to 128 partitions on PE, mult from PSUM
                AMf = AM[:].rearrange("b g f -> b (g f)")
                wflat = wview.rearrange("p g f -> p (g f)")
                p0 = 0
                while p0 < GW:
                    pw = min(512, GW - p0)
                    psS = psp.tile([128, 512], dt.float32, tag="psS", bufs=1,
                                   name="psS")
                    nc.tensor.matmul(psS[:, 0:pw], eye2_sb[:],
                                     AMf[:, p0:p0 + pw], start=True, stop=True)
                    nc.vector.tensor_tensor(wflat[:, p0:p0 + pw],
                                            acc0[:, p0:p0 + pw], psS[:, 0:pw],
                                            ALU.mult)
                    p0 += pw
                if out_dram is not None:
                    nc.sync.dma_start(
                        out_dram[128 * c0:128 * (c0 + G), f0:f0 + fw]
                        .rearrange("(g p) f -> p g f", p=128),
                        wview)

            def make_gu_tasks(si, fg0):
                nfg = cfg.slices[si]
                fw = nfg * 128
                wg, wu = wtiles[si]
                out = []
                for (c0, G) in cfg.slice_groups(nfg):
                    out.append(lambda c0=c0, G=G: emit_deq(
                        g_nib, g_amc, c0, G, fg0 * 128, fw,
                        wg[:, c0:c0 + G, :], cfg.gp_terms))
                    out.append(lambda c0=c0, G=G: emit_deq(
                        u_nib, u_amc, c0, G, fg0 * 128, fw,
                        wu[:, c0:c0 + G, :], cfg.gp_terms))
                return out

            def make_down_tasks():
                out = []
                npc = max(1, D // cfg.deq_w)    # col pieces per chunk
                pw = D // npc
                for c in range(NFG):
                    for h in range(npc):
                        def f(c=c, h=h):
                            wt = dqp.tile([128, 1, pw], dt.bfloat16,
                                          tag="wdt", bufs=1, name="wdt")
                            emit_deq(d_nib, d_amc, c, 1, h * pw, pw, wt[:],
                                     cfg.gp_terms_down, out_dram=wd_dram)
                        out.append(f)
                return out

            n_xh = 2 if cfg.DCH >= 8 else 1
            CH = cfg.DCH // n_xh      # chunks per x-half

            fg0s = np.cumsum([0] + cfg.slices).tolist()
            nsl = len(cfg.slices)

            open_wpool(0)
            tasks.extend(make_gu_tasks(0, fg0s[0]))
            pop_tasks(len(tasks))       # slice 0 dequant upfront
            if nsl > 1:
                open_wpool(1)
                tasks.extend(make_gu_tasks(1, fg0s[1]))
            down_added = nsl <= 1
            if down_added:
                tasks.extend(make_down_tasks())

            for si in range(nsl):
                fg0, nfg = fg0s[si], cfg.slices[si]
                fw = nfg * 128
                wg, wu = wtiles[si]
                quota = (len(tasks) + cfg.NT - 1) // cfg.NT if tasks else 0

                for t in range(cfg.NT):
                    tt = slice(TTW * t, TTW * (t + 1))
                    xth = []
                    for h in range(n_xh):
                        xh = xtp.tile([128, CH, TTW], dt.bfloat16, tag="xt",
                                      bufs=n_xh, name="xh")
                        nc.sync.dma_start(
                            xh[:], xT[128 * CH * h:128 * CH * (h + 1), tt]
                            .rearrange("(c p) t -> p c t", p=128))
                        xth.append(xh)

                    if si == 0:
                        pxag = psp.tile([R, TTW], dt.float32, tag="pxa",
                                        bufs=2, name="pxag")
                        pxau = psp.tile([R, TTW], dt.float32, tag="pxa",
                                        bufs=2, name="pxau")
                        for ci in range(cfg.DCH):
                            nc.tensor.matmul(
                                pxag[:], agu_sb[:, ci, 0:R],
                                xth[ci // CH][:, ci % CH, :],
                                start=(ci == 0), stop=(ci == cfg.DCH - 1))
                        for ci in range(cfg.DCH):
                            nc.tensor.matmul(
                                pxau[:], agu_sb[:, ci, R:2 * R],
                                xth[ci // CH][:, ci % CH, :],
                                start=(ci == 0), stop=(ci == cfg.DCH - 1))
                        nc.scalar.copy(xag_sb[:, tt], pxag[:])
                        nc.scalar.copy(xau_sb[:, tt], pxau[:])

                    x3b = p1p.tile([128, nfg, TTW], dt.bfloat16, tag="x3b",
                                   bufs=2, name="x3b")
                    for fg in range(nfg):
                        fa = slice(128 * (fg0 + fg), 128 * (fg0 + fg + 1))
                        fl = slice(128 * fg, 128 * (fg + 1))
                        pg = psp.tile([128, TTW], dt.float32, tag="pg", bufs=2,
                                      name="pg")
                        pu = psp.tile([128, TTW], dt.float32, tag="pu", bufs=2,
                                      name="pu")
                        for ci in range(cfg.DCH):
                            nc.tensor.matmul(pg[:], wg[:, ci, fl],
                                             xth[ci // CH][:, ci % CH, :],
                                             start=(ci == 0), stop=False)
                        nc.tensor.matmul(pg[:], bg_sb[:, fa], xag_sb[:, tt],
                                         start=False, stop=True)
                        for ci in range(cfg.DCH):
                            nc.tensor.matmul(pu[:], wu[:, ci, fl],
                                             xth[ci // CH][:, ci % CH, :],
                                             start=(ci == 0), stop=False)
                        nc.tensor.matmul(pu[:], bu_sb[:, fa], xau_sb[:, tt],
                                         start=False, stop=True)
                        pub = p1p.tile([128, TTW], dt.bfloat16, tag="pub",
                                       bufs=2, name="pub")
                        nc.scalar.copy(pub[:], pu[:])
                        if cfg.use_silu:
                            nc.scalar.activation(x3b[:, fg, :], pg[:],
                                                 AFT.Silu)
                            nc.gpsimd.tensor_tensor(x3b[:, fg, :],
                                                    x3b[:, fg, :], pub[:],
                                                    ALU.mult)
                        else:
                            sg = p1p.tile([128, TTW], dt.bfloat16, tag="sg",
                                          bufs=2, name="sg")
                            nc.scalar.activation(sg[:], pg[:], AFT.Sigmoid)
                            pgb = p1p.tile([128, TTW], dt.bfloat16, tag="pgb",
                                           bufs=2, name="pgb")
                            nc.scalar.copy(pgb[:], pg[:])
                            nc.gpsimd.tensor_tensor(sg[:], sg[:], pgb[:],
                                                    ALU.mult)
                            nc.gpsimd.tensor_tensor(x3b[:, fg, :], sg[:],
                                                    pub[:], ALU.mult)
                    nc.sync.dma_start(
                        x3_dram[128 * fg0:128 * fg0 + fw, tt]
                        .rearrange("(g p) t -> p g t", p=128),
                        x3b[:])
                    pop_tasks(quota)

                # queue what dequants next
                if si + 2 < nsl:
                    open_wpool(si + 2)
                    tasks.extend(make_gu_tasks(si + 2, fg0s[si + 2]))
                elif not down_added:
                    down_added = True
                    tasks.extend(make_down_tasks())

            pop_tasks(len(tasks))       # down-weight dequant tail

            # phase-2 prologue: x3a = Ad^T @ x3 (overlaps the dequant tail)
            for tg2 in range(cfg.NTG):
                tsl = slice(128 * tg2, 128 * (tg2 + 1))
                x3p = p1p.tile([128, NFG, 128], dt.bfloat16, tag="x3b",
                               bufs=2, name="x3p")
                nc.sync.dma_start(
                    x3p[:], x3_dram[:, tsl].rearrange("(c p) t -> p c t",
                                                      p=128))
                px3a = psp.tile([R, 128], dt.float32, tag="px3a", name="px3a")
                for ci in range(NFG):
                    nc.tensor.matmul(px3a[:], ad_sb[:, ci, :], x3p[:, ci, :],
                                     start=(ci == 0), stop=(ci == NFG - 1))
                nc.scalar.copy(x3a_sb[:, tsl], px3a[:])

        dqp_cm.__exit__(None, None, None)

        # ------------- phase 2 -------------
        with tc.tile_pool(name="p2", bufs=1) as p2p, \
             tc.tile_pool(name="wd", bufs=1) as wdp, \
             tc.tile_pool(name="ps2", bufs=1, space="PSUM") as ps2:
            bd_sb = p2p.tile([R, D], dt.bfloat16, tag="bd", name="bd_sb")
            nc.sync.dma_start(bd_sb[:], b_d[:])
            n_dj = cfg.DH // 512
            for dh in range(cfg.n_dh):
                dsl = slice(cfg.DH * dh, cfg.DH * (dh + 1))
                wd_sb = wdp.tile([128, NFG, cfg.DH], dt.bfloat16, tag="wd",
                                 bufs=min(2, cfg.n_dh), name="wd_sb")
                nc.sync.dma_start(
                    wd_sb[:], wd_dram[:, dsl].rearrange("(c p) d -> p c d",
                                                        p=128))
                for q in range(cfg.n_q):
                    j = (dh * cfg.n_q + q) % 2
                    for tgl in range(cfg.TQ // 128):
                        tg = (cfg.TQ // 128) * q + tgl
                        tsl = slice(128 * tg, 128 * (tg + 1))
                        x3g = p2p.tile([128, NFG, 128], dt.bfloat16, tag="x3g",
                                       bufs=3, name="x3g")
                        nc.sync.dma_start(
                            x3g[:], x3_dram[:, tsl]
                            .rearrange("(c p) t -> p c t", p=128))
                        pds = [ps2.tile([128, 512], dt.float32, tag="pd",
                                        bufs=8, name=f"pd{dj}")
                               for dj in range(n_dj)]
                        for ci in range(NFG):
                            for dj in range(n_dj):
                                nc.tensor.matmul(
                                    pds[dj][:], x3g[:, ci, :],
                                    wd_sb[:, ci, 512 * dj:512 * (dj + 1)],
                                    start=(ci == 0), stop=False)
                        for dj in range(n_dj):
                            nc.tensor.matmul(
                                pds[dj][:], x3a_sb[:, tsl],
                                bd_sb[:, cfg.DH * dh + 512 * dj:
                                      cfg.DH * dh + 512 * (dj + 1)],
                                start=False, stop=True)
                        yb = p2p.tile([128, cfg.DH], dt.bfloat16, tag="yb",
                                      bufs=2, name="yb")
                        for dj in range(n_dj):
                            nc.scalar.copy(yb[:, 512 * dj:512 * (dj + 1)],
                                           pds[dj][:])
                        nc.sync.dma_start(
                            rs_in[j][128 * tgl:128 * (tgl + 1), :], yb[:])
                    nc.gpsimd.collective_compute(
                        "ReduceScatter", ALU.add, replica_groups=rg,
                        ins=[rs_in[j][:, :].opt()],
                        outs=[rs_out[j][:, :].opt()],
                    )
                    # convert + emit this quarter's output rows on DVE (so the
                    # ACT stream never blocks on the collective)
                    for r0 in range(0, cfg.TQC, 128):
                        rw = min(128, cfg.TQC - r0)
                        rt = p2p.tile([128, cfg.DH], dt.bfloat16, tag="rt",
                                      bufs=2, name="rt")
                        nc.sync.dma_start(rt[0:rw, :],
                                          rs_out[j][r0:r0 + rw, :])
                        yf = p2p.tile([128, cfg.DH], dt.float32, tag="yf",
                                      bufs=2, name="yf")
                        nc.vector.tensor_scalar(yf[0:rw, :], rt[0:rw, :], 1.0,
                                                None, ALU.mult)
                        nc.sync.dma_start(
                            y_out[cfg.TQC * q + r0:cfg.TQC * q + r0 + rw, dsl],
                            yf[0:rw, :])

    nc.compile()
    return nc


# ----------------- host side -----------------

_CACHE = {}


def _get_graph(cfg: Cfg):
    key = (cfg.D, cfg.T, cfg.F, cfg.ncores, cfg.use_silu)
    if key not in _CACHE:
        _CACHE[key] = build_graph(cfg)
    return _CACHE[key]


def _prep_inputs(cfg: Cfg, inputs):
    """Shard + lay out the full inputs for each core (marshalling only:
    transpose, nibble unpack, dtype casts, padding)."""
    D, T, F, FP, FS, R = cfg.D, cfg.T, cfg.F, cfg.FP, cfg.FS, cfg.R
    blk = cfg.block

    x = np.asarray(inputs["x"])
    xT = np.ascontiguousarray(x.T).astype(BF16)

    def nib_split(packed, rows, cols):
        """packed int32 words (one byte each) -> u8 nibble values [rows, cols]."""
        b = (np.asarray(packed).astype(np.int64) & 0xFF).astype(np.uint8)
        b = b.reshape(rows, cols // 2)
        out = np.empty((rows, cols), np.uint8)
        out[:, 0::2] = b >> 4
        out[:, 1::2] = b & 0xF
        return out

    # gate/up: [F, D] -> pad rows to FP -> transpose -> [D, FP]; shard cols
    def prep_gu(packed, absmax):
        nib = nib_split(packed, F, D)
        nib = np.concatenate([nib, np.zeros((FP - F, D), np.uint8)], 0)
        nibT = np.ascontiguousarray(nib.T)              # [D, FP]
        am = np.asarray(absmax, np.float32).reshape(F, D // blk)
        am = np.concatenate([am, np.zeros((FP - F, D // blk), np.float32)], 0)
        amT = np.ascontiguousarray(am.T).astype(BF16)   # [D/blk, FP]
        return nibT, amT

    g_nibT, g_amT = prep_gu(inputs["w_gate_packed"], inputs["w_gate_absmax"])
    u_nibT, u_amT = prep_gu(inputs["w_up_packed"], inputs["w_up_absmax"])

    # down: [D, F] -> pad cols to FP -> transpose -> [FP, D]; shard rows
    d_nib = nib_split(inputs["w_down_packed"], D, F)
    d_nib = np.concatenate([d_nib, np.zeros((D, FP - F), np.uint8)], 1)
    d_nibT = np.ascontiguousarray(d_nib.T)              # [FP, D]
    d_am = np.asarray(inputs["w_down_absmax"], np.float32).reshape(D, F // blk)
    d_am = np.concatenate([d_am, np.zeros((D, (FP - F) // blk), np.float32)], 1)
    d_amT = np.ascontiguousarray(d_am.T).astype(BF16)   # [FP/blk, D]

    code_rep = np.broadcast_to(
        np.asarray(inputs["code"]).astype(np.float32)[None, :], (128, 16)
    ).copy()
    a_gu = np.concatenate(
        [np.asarray(inputs["w_gate_lora_a"]),
         np.asarray(inputs["w_up_lora_a"])], axis=1).astype(BF16)

    def pad_cols(m):
        return np.concatenate([m, np.zeros((m.shape[0], FP - F), m.dtype)], 1)

    b_g_full = pad_cols(np.asarray(inputs["w_gate_lora_b"], np.float32))
    b_u_full = pad_cols(np.asarray(inputs["w_up_lora_b"], np.float32))
    a_d_full = np.concatenate(
        [np.asarray(inputs["w_down_lora_a"], np.float32),
         np.zeros((FP - F, R), np.float32)], 0)
    b_d = np.asarray(inputs["w_down_lora_b"]).astype(BF16)

    eye2 = np.zeros((cfg.ABC, 128), BF16)
    for b in range(cfg.ABC):
        eye2[b, b * blk:(b + 1) * blk] = 1.0

    in_maps = []
    nab_s = FS // blk
    for i in range(cfg.ncores):
        fsl = slice(FS * i, FS * (i + 1))
        in_maps.append({
            "xT": xT,
            "g_nib": np.ascontiguousarray(g_nibT[:, fsl]),
            "u_nib": np.ascontiguousarray(u_nibT[:, fsl]),
            "d_nib": np.ascontiguousarray(d_nibT[fsl]),
            "g_amc": np.ascontiguousarray(g_amT[:, fsl]),
            "u_amc": np.ascontiguousarray(u_amT[:, fsl]),
            "d_amc": np.ascontiguousarray(d_amT[nab_s * i:nab_s * (i + 1)]),
            "code_rep": code_rep,
            "a_gu": a_gu,
            "b_g": np.ascontiguousarray(b_g_full[:, fsl]).astype(BF16),
            "b_u": np.ascontiguousarray(b_u_full[:, fsl]).astype(BF16),
            "a_d": np.ascontiguousarray(a_d_full[fsl]).astype(BF16),
            "b_d": b_d,
            "eye2": eye2,
        })
    return in_maps


def _gather(cfg: Cfg, results):
    """Reassemble full [T, D] output from per-core quarter-row blocks."""
    y = np.empty((cfg.T, cfg.D), np.float32)
    for i in range(cfg.ncores):
        yi = results[i]["y_out"]
        for q in range(cfg.n_q):
            r0 = cfg.TQ * q + cfg.TQC * i
            y[r0:r0 + cfg.TQC] = yi[cfg.TQC * q:cfg.TQC * (q + 1)]
    return y


def run(cfg: Cfg, inputs, trace=False, **kwargs):
    nc = _get_graph(cfg)
    in_maps = _prep_inputs(cfg, inputs)
    res = run_bass_kernel_spmd(
        nc, in_maps, core_ids=list(range(cfg.ncores)), trace=trace, **kwargs
    )
    y = _gather(cfg, res.results)
    return y, res


def kernel(**inputs) -> np.ndarray:
    cfg = Cfg()
    y, _ = run(cfg, inputs)
    return y.astype(np.float32)
